# revision 28
# baseline (speedup 1.0000x reference)
"""GravityAE GNN message-passing kernel for 8 TRN2 NeuronCores (Bass/Tile).

Algorithm (GCN autoencoder, BN folded into W/shift):
  scale_k = gamma_k / sqrt(var_k + eps); shift_k = beta_k + (b_k - mean_k)*scale_k
  W1p = W1 * scale1; W2p = W2 * scale2
  dinv[n] = 1/sqrt(in_degree incl self loop)
  xs1 = dinv * (x @ W1p)                      (bf16 node table, gathered by src)
  h'  = dinv * leaky(dinv[d]*segsum_d(xs1[src]) + shift1)
  z   = leaky(dinv[d]*(segsum_d(h'[src]) @ W2p) + shift2)
  out[e] = sigmoid(z[dst,64] - ||z[src,:64] - z[dst,:64]||)

Distribution: everything is dst-window sharded (each core owns 49 contiguous
128-node windows; edges+self-loops sorted by dst).  Per window the source
rows are fetched with batched dma_gather of node-PAIRS (idx = src>>1 int16;
pair rows satisfy the 256B elem-size constraint and the int16 index range,
<=1024 idxs per instruction -- Q7 firmware cap).  Aggregation is an even/odd
dual indicator matmul accumulated in PSUM (S_even/S_odd one-hot matrices from
a single is_equal over a host-interleaved dstfEO table).  Layer 2 aggregates
h' first and applies W2 after (the transform commutes with segment-sum),
which keeps every gather at 128 bf16 columns.  Three bf16 AllGathers rebuild
the full node tables between stages.  Decode is dst-window local: src
positions are pair-gathered from the z-pos table with a copy_predicated
even/odd select; dst position+mass come from the local window via a
transposed one-hot matmul (one-hot built from a broadcast-DMA'd dst row);
the host un-permutes the dst-sorted edge outputs.
"""
import numpy as np

P = 128
EPS = 1e-5


def _blob_layout(NWc, C_max, F1, F2):
    """Byte-column layout of the packed small-table parameter (per partition).
    f32 tables first (4B alignment), then bf16, then u8; total padded even."""
    off, lay = 0, {}
    for name, nbytes in [
        ("sh1", F1 * 4), ("sh2", F2 * 4), ("dinv", NWc * 4),
        ("w1", F1 * 2), ("w2", F2 * 2),
        ("dstfEO", NWc * 2 * C_max * 2), ("oddf", NWc * C_max),
    ]:
        lay[name] = (off, off + nbytes)
        off += nbytes + (nbytes & 1)
    off = (off + 3) // 4 * 4
    return lay, off


# --------------------------------------------------------------------------
# host-side preprocessing
# --------------------------------------------------------------------------
def _build_host_tables(N, E, src, dst, n_cores):
    import ml_dtypes

    NW = ((N + P - 1) // P + n_cores - 1) // n_cores * n_cores
    NP_ = NW * P
    NWc = NW // n_cores
    s_all = np.concatenate([src, np.arange(N, dtype=np.int64)])
    d_all = np.concatenate([dst, np.arange(N, dtype=np.int64)])
    M = E + N

    deg = np.bincount(d_all, minlength=NP_).astype(np.float64)
    dinv = np.zeros(NP_, np.float32)
    nz = deg > 0
    dinv[nz] = (1.0 / np.sqrt(deg[nz])).astype(np.float32)

    order = np.argsort(d_all, kind="stable")
    s_sorted = s_all[order]
    d_sorted = d_all[order]
    win = (d_sorted // P).astype(np.int64)
    counts = np.bincount(win, minlength=NW)
    C_max = max(1, int(np.ceil(counts.max() / P)))
    CW = C_max * P
    starts = np.zeros(NW + 1, np.int64)
    np.cumsum(counts, out=starts[1:])

    k = np.arange(M) - starts[win]            # slot within window
    pairidx = np.zeros((NW, CW), np.int16)
    oddf = np.zeros((NW, CW), np.uint8)
    dstf = np.full((NW, CW), -1.0, ml_dtypes.bfloat16)
    pairidx[win, k] = (s_sorted >> 1).astype(np.int16)
    oddf[win, k] = (s_sorted & 1).astype(np.uint8)
    dstf[win, k] = (d_sorted - win * P).astype(ml_dtypes.bfloat16)
    isodd = oddf.astype(bool)
    dstfE = np.where(isodd, ml_dtypes.bfloat16(-1.0), dstf)
    dstfO = np.where(isodd, dstf, ml_dtypes.bfloat16(-1.0))

    # per-core tables
    KW = CW // 16
    idx16 = np.empty((n_cores, 16, NWc * KW), np.int16)
    oddf_c = np.empty((n_cores, P, NWc * C_max), np.uint8)
    dstfEO_c = np.empty((n_cores, P, NWc * 2 * C_max), ml_dtypes.bfloat16)
    dstfT_c = np.empty((n_cores, NWc, CW), ml_dtypes.bfloat16)

    def _slotize(tab, c):
        return (tab[c * NWc:(c + 1) * NWc].reshape(NWc, C_max, P)
                .transpose(2, 0, 1).reshape(P, NWc, C_max))

    for c in range(n_cores):
        blk = pairidx[c * NWc:(c + 1) * NWc]          # [NWc, CW]
        idx16[c] = (blk.reshape(NWc, KW, 16).transpose(2, 0, 1)
                    .reshape(16, NWc * KW))
        oddf_c[c] = _slotize(oddf, c).reshape(P, NWc * C_max)
        dstfEO_c[c] = np.concatenate(
            [_slotize(dstfE, c), _slotize(dstfO, c)], axis=2
        ).reshape(P, NWc * 2 * C_max)
        dstfT_c[c] = dstf[c * NWc:(c + 1) * NWc]

    cws = tuple(
        int(np.ceil(counts.reshape(n_cores, NWc).max(axis=0)[wl] / P)) or 1
        for wl in range(NWc))

    # output unpermute: sorted entry j -> (core, row, col); keep real edges
    core_of = win // NWc
    wl = win % NWc
    col = wl * C_max + (k // P)
    row = k % P
    orig = order
    real = orig < E
    out_map = (orig[real], core_of[real], row[real], col[real])

    return dict(N=N, E=E, NW=NW, NP=NP_, NWc=NWc, C_max=C_max, CW=CW,
                cws=cws, dinv=dinv, idx16=idx16, oddf_c=oddf_c,
                dstfEO_c=dstfEO_c, dstfT_c=dstfT_c, out_map=out_map)


# --------------------------------------------------------------------------
# bass program
# --------------------------------------------------------------------------
def _build_program(NP_, NWc, C_max, F1, F2, n_cores, cws=None, stages="ABCD"):
    import concourse.bass as bass
    import concourse.tile as tile
    from concourse import bacc, mybir

    dt = mybir.dt
    f32 = dt.float32
    bf16 = dt.bfloat16
    Nc = NWc * P
    CW = C_max * P
    KW = CW // 16
    Fp = F2 - 1                                 # position dims (64)
    if cws is None:
        cws = (C_max,) * NWc
    af = mybir.ActivationFunctionType
    op = mybir.AluOpType

    nc = bacc.Bacc("TRN2", target_bir_lowering=False, debug=False,
                   num_devices=n_cores)
    lay, TOT = _blob_layout(NWc, C_max, F1, F2)
    xT_in = nc.declare_dram_parameter("xT", [P, Nc], bf16, isOutput=False)
    blob_in = nc.declare_dram_parameter("blob", [P, TOT], dt.uint8, isOutput=False)
    idx_in = nc.declare_dram_parameter("idx16", [16, NWc * KW], dt.int16, isOutput=False)
    dstfT_in = nc.declare_dram_parameter("dstfT", [NWc, CW], bf16, isOutput=False)
    out_dram = nc.declare_dram_parameter("out", [P, NWc * C_max], f32, isOutput=True)

    def _bslice(name, dtype):
        a, b = lay[name]
        return blob_in[:, a:b].bitcast(dtype)

    rg = [list(range(n_cores))]

    with tile.TileContext(nc) as tc:
        with (
            tc.tile_pool(name="const", bufs=1) as cpool,
            tc.tile_pool(name="sbuf", bufs=2) as pool,
            tc.tile_pool(name="psA", bufs=2, space="PSUM") as psA,
            tc.tile_pool(name="psZ", bufs=1, space="PSUM") as psZ,
            tc.tile_pool(name="psD", bufs=1, space="PSUM") as psD,
            tc.tile_pool(name="dram", bufs=1, space="DRAM") as dpool,
        ):
            # ---- constants ----
            w1_t = cpool.tile([F1, F1], bf16)
            w2_t = cpool.tile([F1, F2], bf16)
            sh1_t = cpool.tile([P, F1], f32)
            sh2_t = cpool.tile([P, F2], f32)
            xT_t = cpool.tile([P, Nc], bf16)
            dinv_t = cpool.tile([P, NWc], f32)
            idx_t = cpool.tile([P, NWc * KW], dt.int16)
            odd_t = cpool.tile([P, NWc * C_max], dt.uint8)
            dstfEO_t = cpool.tile([P, NWc * 2 * C_max], bf16)
            nc.sync.dma_start(out=w1_t[:], in_=_bslice("w1", bf16))
            nc.sync.dma_start(out=w2_t[:], in_=_bslice("w2", bf16))
            nc.sync.dma_start(out=sh1_t[:], in_=_bslice("sh1", f32))
            nc.sync.dma_start(out=sh2_t[:], in_=_bslice("sh2", f32))
            nc.sync.dma_start(out=xT_t[:], in_=xT_in[:])
            nc.sync.dma_start(out=dinv_t[:], in_=_bslice("dinv", f32))
            nc.sync.dma_start(out=idx_t[0:16, :], in_=idx_in[:])
            for _k in range(1, 8):
                nc.sync.dma_start(out=idx_t[16 * _k:16 * (_k + 1), :],
                                  in_=idx_t[0:16, :])
            nc.sync.dma_start(out=odd_t[:], in_=_bslice("oddf", dt.uint8))
            nc.sync.dma_start(out=dstfEO_t[:], in_=_bslice("dstfEO", bf16))

            # device-built iotas
            iota_i = cpool.tile([P, 2 * C_max, P], dt.int16)
            nc.gpsimd.iota(iota_i[:], pattern=[[0, 2 * C_max], [1, P]], base=0,
                           channel_multiplier=0)
            iota_t = cpool.tile([P, 2 * C_max, P], bf16)
            nc.vector.tensor_copy(iota_t[:], iota_i[:])
            iotaP_i = cpool.tile([P, CW], dt.int16)
            nc.gpsimd.iota(iotaP_i[:], pattern=[[0, CW]], base=0,
                           channel_multiplier=1)
            iotaP_t = cpool.tile([P, CW], bf16)
            nc.vector.tensor_copy(iotaP_t[:], iotaP_i[:])

            posmj_t = cpool.tile([P, NWc * P], bf16)
            nc.vector.memset(posmj_t[:], 0.0)
            stage_d2 = cpool.tile([P, NWc * C_max], f32)
            stage_mj = cpool.tile([P, NWc * C_max], f32)
            nc.vector.memset(stage_d2[:], 0.0)
            nc.vector.memset(stage_mj[:], 0.0)

            # ---- collective buffers (pair-packed views) ----
            ag1 = dpool.tile([Nc, F1], bf16)
            xs1_full = dpool.tile([NP_ // 2, 2 * F1], bf16, addr_space="Shared")
            ag2 = dpool.tile([Nc, F1], bf16)
            h_full = dpool.tile([NP_ // 2, 2 * F1], bf16, addr_space="Shared")
            ag3 = dpool.tile([Nc, Fp], bf16)
            pos_full = dpool.tile([NP_ // 2, 2 * Fp], bf16, addr_space="Shared")

            # ---- stage A: xs1 shard = dinv * (x @ W1p) ----
            for w in range(NWc):
                ps = psA.tile([P, F1], f32, tag="mm")
                nc.tensor.matmul(ps[:], xT_t[:, w * P:(w + 1) * P], w1_t[:],
                                 start=True, stop=True)
                xs = pool.tile([P, F1], bf16, tag="axs")
                nc.scalar.activation(xs[:], ps[:], af.Copy,
                                     scale=dinv_t[:, w:w + 1])
                nc.sync.dma_start(out=ag1[w * P:(w + 1) * P, :], in_=xs[:])

            nc.gpsimd.collective_compute(
                "AllGather", mybir.AluOpType.bypass,
                ins=[ag1.opt()], outs=[xs1_full.opt()], replica_groups=rg)

            # ---- shared gather+select+S helper ----
            GMAX = 8                     # chunks per dma_gather (Q7 ~1024-idx cap)

            def gather_pairs(full_tab, elem, w, tagp, sel=False, half=0):
                Cw = cws[w]
                msg = pool.tile([P, C_max, elem], bf16, tag=tagp + "msg")
                for g0 in range(0, Cw, GMAX):
                    g1 = min(g0 + GMAX, Cw)
                    ni = (g1 - g0) * P
                    nc.gpsimd.dma_gather(
                        out_ap=msg[:, g0:g1, :], in_ap=full_tab[:],
                        idxs_ap=idx_t[:, w * KW + g0 * 8:w * KW + g1 * 8],
                        num_idxs=ni, num_idxs_reg=ni, elem_size=elem)
                if sel:
                    nc.vector.copy_predicated(
                        msg[:, 0:Cw, 0:half],
                        odd_t[:, w * C_max:w * C_max + Cw]
                        .rearrange("p (c o) -> p c o", o=1)
                        .to_broadcast([P, Cw, half]),
                        msg[:, 0:Cw, half:2 * half])
                return msg

            def build_S_EO(w, tag):
                Cw = cws[w]
                span = C_max + Cw          # E-half padded to C_max, O-half Cw
                S = pool.tile([P, 2 * C_max, P], bf16, tag=tag)
                base = w * 2 * C_max
                nc.vector.tensor_tensor(
                    out=S[:, 0:span, :],
                    in0=dstfEO_t[:, base:base + span]
                    .rearrange("p (c o) -> p c o", o=1)
                    .to_broadcast([P, span, P]),
                    in1=iota_t[:, 0:span, :], op=op.is_equal)
                return S

            # ---- stage B: h' windows ----
            for w in range(NWc if "B" in stages else 0):
                msg = gather_pairs(xs1_full, 2 * F1, w, "b")
                S = build_S_EO(w, "S")
                ps = psA.tile([P, F1], f32, tag="mm")
                Cw = cws[w]
                for c in range(Cw):
                    nc.tensor.matmul(ps[:], S[:, c, :], msg[:, c, 0:F1],
                                     start=(c == 0), stop=False)
                    nc.tensor.matmul(ps[:], S[:, C_max + c, :], msg[:, c, F1:2 * F1],
                                     start=False, stop=(c == Cw - 1))
                t = pool.tile([P, F1], f32, tag="bt")
                nc.vector.scalar_tensor_tensor(
                    out=t[:], in0=ps[:], scalar=dinv_t[:, w:w + 1], in1=sh1_t[:],
                    op0=op.mult, op1=op.add)
                h = pool.tile([P, F1], f32, tag="bh")
                nc.vector.scalar_tensor_tensor(
                    out=h[:], in0=t[:], scalar=0.1, in1=t[:],
                    op0=op.mult, op1=op.max)
                hp = pool.tile([P, F1], bf16, tag="bhp")
                nc.scalar.activation(hp[:], h[:], af.Copy,
                                     scale=dinv_t[:, w:w + 1])
                nc.sync.dma_start(out=ag2[w * P:(w + 1) * P, :], in_=hp[:])

            nc.gpsimd.collective_compute(
                "AllGather", mybir.AluOpType.bypass,
                ins=[ag2.opt()], outs=[h_full.opt()], replica_groups=rg)

            # ---- stage C: z windows (aggregate then transform) ----
            for w in range(NWc if "C" in stages else 0):
                msg = gather_pairs(h_full, 2 * F1, w, "c")
                S = build_S_EO(w, "S")
                psT = psA.tile([P, F1], f32, tag="mm")
                Cw = cws[w]
                for c in range(Cw):
                    nc.tensor.matmul(psT[:], msg[:, c, 0:F1], S[:, c, :],
                                     start=(c == 0), stop=False)
                    nc.tensor.matmul(psT[:], msg[:, c, F1:2 * F1], S[:, C_max + c, :],
                                     start=False, stop=(c == Cw - 1))
                aggb = pool.tile([P, F1], bf16, tag="cagg")
                nc.scalar.copy(aggb[:], psT[:])
                psz = psZ.tile([P, F2], f32, tag="z")
                nc.tensor.matmul(psz[:], aggb[:], w2_t[:], start=True, stop=True)
                tz = pool.tile([P, F2], f32, tag="ct")
                nc.vector.scalar_tensor_tensor(
                    out=tz[:], in0=psz[:], scalar=dinv_t[:, w:w + 1], in1=sh2_t[:],
                    op0=op.mult, op1=op.add)
                z = pool.tile([P, F2], f32, tag="cz")
                nc.vector.scalar_tensor_tensor(
                    out=z[:], in0=tz[:], scalar=0.1, in1=tz[:],
                    op0=op.mult, op1=op.max)
                nc.vector.tensor_copy(posmj_t[:, w * P:w * P + F2], z[:])
                nc.sync.dma_start(out=ag3[w * P:(w + 1) * P, :],
                                  in_=posmj_t[:, w * P:w * P + Fp])

            nc.gpsimd.collective_compute(
                "AllGather", mybir.AluOpType.bypass,
                ins=[ag3.opt()], outs=[pos_full.opt()], replica_groups=rg)

            # ---- stage D: decode (dst-window local) ----
            for w in range(NWc if "D" in stages else 0):
                Cw = cws[w]
                msg = gather_pairs(pos_full, 2 * Fp, w, "d", sel=True, half=Fp)
                dstrep = pool.tile([P, CW], bf16, tag="drep")
                nc.sync.dma_start(out=dstrep[:, 0:Cw * P],
                                  in_=dstfT_in[w:w + 1, 0:Cw * P]
                                  .to_broadcast([P, Cw * P]))
                ST = pool.tile([P, CW], bf16, tag="dST")
                nc.vector.tensor_tensor(out=ST[:, 0:Cw * P],
                                        in0=dstrep[:, 0:Cw * P],
                                        in1=iotaP_t[:, 0:Cw * P],
                                        op=op.is_equal)
                psd = psD.tile([P, C_max, P], f32, tag="dall")
                for c in range(Cw):
                    nc.tensor.matmul(psd[:, c, :], ST[:, c * P:(c + 1) * P],
                                     posmj_t[:, w * P:(w + 1) * P],
                                     start=True, stop=True)
                diff = pool.tile([P, C_max, Fp], bf16, tag="ddiff")
                nc.vector.tensor_tensor(out=diff[:, 0:Cw, :],
                                        in0=msg[:, 0:Cw, 0:Fp],
                                        in1=psd[:, 0:Cw, 0:Fp], op=op.subtract)
                sq = pool.tile([P, C_max, Fp], f32, tag="dsq")
                nc.scalar.square(sq[:, 0:Cw, :], diff[:, 0:Cw, :])
                nc.vector.reduce_sum(
                    out=stage_d2[:, w * C_max:w * C_max + Cw]
                    .rearrange("p (c o) -> p c o", o=1),
                    in_=sq[:, 0:Cw, :], axis=mybir.AxisListType.X)
                nc.scalar.copy(stage_mj[:, w * C_max:w * C_max + Cw],
                               psd[:, 0:Cw, Fp])

            # ---- finale: sigmoid(mj - sqrt(d2)) ----
            sd = cpool.tile([P, NWc * C_max], f32)
            nc.scalar.activation(sd[:], stage_d2[:], af.Sqrt)
            sv = cpool.tile([P, NWc * C_max], f32)
            nc.vector.tensor_tensor(out=sv[:], in0=stage_mj[:], in1=sd[:],
                                    op=op.subtract)
            so = cpool.tile([P, NWc * C_max], f32)
            nc.scalar.activation(so[:], sv[:], af.Sigmoid)
            nc.sync.dma_start(out=out_dram[:], in_=so[:])
    nc.compile()
    return nc


_PROG_CACHE = {}
_EXEC_CACHE = {}


def _run_cached(nc, in_maps, n_cores):
    """Execute `nc` via PJRT with a cached jitted executable (the library
    path re-jits a fresh closure every call). Mirrors bass2jax.run_bass_via_pjrt."""
    import jax
    import jax.numpy as jnp
    from jax.sharding import Mesh, PartitionSpec
    from jax.experimental.shard_map import shard_map
    from concourse import bass2jax, mybir
    from concourse.bass2jax import _bass_exec_p, install_neuronx_cc_hook, partition_id_tensor

    key = id(nc)
    if key not in _EXEC_CACHE:
        install_neuronx_cc_hook()
        partition_name = (nc.partition_id_tensor.name
                          if nc.partition_id_tensor else None)
        in_names, out_names, out_avals = [], [], []
        for alloc in nc.m.functions[0].allocations:
            if not isinstance(alloc, mybir.MemoryLocationSet):
                continue
            name = alloc.memorylocations[0].name
            if alloc.kind == "ExternalInput":
                if name != partition_name:
                    in_names.append(name)
            elif alloc.kind == "ExternalOutput":
                out_names.append(name)
                out_avals.append(jax.core.ShapedArray(
                    tuple(alloc.tensor_shape), mybir.dt.np(alloc.dtype)))
        n_params = len(in_names)
        all_names = in_names + out_names + ([partition_name] if partition_name else [])
        donate = tuple(range(n_params, n_params + len(out_names)))

        def _body(*args):
            operands = list(args)
            if partition_name is not None:
                operands.append(partition_id_tensor())
            return tuple(_bass_exec_p.bind(
                *operands, out_avals=tuple(out_avals), in_names=tuple(all_names),
                out_names=tuple(out_names), lowering_input_output_aliases=(),
                sim_require_finite=True, sim_require_nnan=True, nc=nc))

        devices = jax.devices()[:n_cores]
        mesh = Mesh(np.asarray(devices), ("core",))
        nio = n_params + len(out_names)
        sharded = jax.jit(
            shard_map(_body, mesh=mesh,
                      in_specs=(PartitionSpec("core"),) * nio,
                      out_specs=(PartitionSpec("core"),) * len(out_names),
                      check_rep=False),
            donate_argnums=donate, keep_unused=True)
        _EXEC_CACHE[key] = (sharded, in_names, out_names, out_avals)

    sharded, in_names, out_names, out_avals = _EXEC_CACHE[key]
    concat_in = [np.concatenate([np.asarray(in_maps[c][n]) for c in range(n_cores)],
                                axis=0) for n in in_names]
    concat_zeros = [np.zeros((n_cores * a.shape[0], *a.shape[1:]), a.dtype)
                    for a in out_avals]
    out_arrs = sharded(*concat_in, *concat_zeros)
    return [{n: np.asarray(out_arrs[i]).reshape(n_cores, *out_avals[i].shape)[c]
             for i, n in enumerate(out_names)} for c in range(n_cores)]


def _get_program(NP_, NWc, C_max, F1, F2, n_cores, cws):
    key = (NP_, NWc, C_max, F1, F2, n_cores, cws)
    if key not in _PROG_CACHE:
        _PROG_CACHE[key] = _build_program(NP_, NWc, C_max, F1, F2, n_cores,
                                          cws=cws)
    return _PROG_CACHE[key]


# --------------------------------------------------------------------------
# public entry
# --------------------------------------------------------------------------
def kernel(x, edge_index, W1, b1, gamma1, beta1, mean1, var1,
           W2, b2, gamma2, beta2, mean2, var2, n_cores=8, _trace=False):
    import ml_dtypes

    x = np.asarray(x, np.float32)
    edge_index = np.asarray(edge_index)
    N, F1 = x.shape
    E = edge_index.shape[1]
    F2 = np.asarray(W2).shape[1]
    src = edge_index[0].astype(np.int64)
    dst = edge_index[1].astype(np.int64)

    ht = _build_host_tables(N, E, src, dst, n_cores)
    NP_, NWc, C_max = ht["NP"], ht["NWc"], ht["C_max"]
    Nc = NWc * P

    scale1 = np.asarray(gamma1) / np.sqrt(np.asarray(var1) + EPS)
    shift1 = (np.asarray(beta1) + (np.asarray(b1) - np.asarray(mean1)) * scale1).astype(np.float32)
    W1p = (np.asarray(W1) * scale1[None, :]).astype(ml_dtypes.bfloat16)
    scale2 = np.asarray(gamma2) / np.sqrt(np.asarray(var2) + EPS)
    shift2 = (np.asarray(beta2) + (np.asarray(b2) - np.asarray(mean2)) * scale2).astype(np.float32)
    W2p = (np.asarray(W2) * scale2[None, :]).astype(ml_dtypes.bfloat16)

    xp = np.zeros((NP_, F1), np.float32)
    xp[:N] = x
    sh1_rep = np.broadcast_to(shift1[None, :], (P, F1)).copy()
    sh2_rep = np.broadcast_to(shift2[None, :], (P, F2)).copy()

    lay, TOT = _blob_layout(NWc, C_max, F1, F2)

    def _pack(**tabs):
        blob = np.zeros((P, TOT), np.uint8)
        for name, arr in tabs.items():
            a, b = lay[name]
            blob[:, a:b] = np.ascontiguousarray(arr).view(np.uint8).reshape(P, -1)
        return blob

    in_maps = []
    for c in range(n_cores):
        xc = xp[c * Nc:(c + 1) * Nc]
        in_maps.append({
            "xT": np.ascontiguousarray(xc.T).astype(ml_dtypes.bfloat16),
            "blob": _pack(
                sh1=sh1_rep, sh2=sh2_rep,
                dinv=np.ascontiguousarray(
                    ht["dinv"][c * Nc:(c + 1) * Nc].reshape(NWc, P).T),
                w1=np.broadcast_to(W1p, (P, F1)).copy() if W1p.shape[0] != P else W1p,
                w2=W2p, dstfEO=ht["dstfEO_c"][c], oddf=ht["oddf_c"][c]),
            "idx16": ht["idx16"][c],
            "dstfT": ht["dstfT_c"][c],
        })

    nc = _get_program(NP_, NWc, C_max, F1, F2, n_cores, ht["cws"])
    import time as _time
    t0 = _time.time()
    results = _run_cached(nc, in_maps, n_cores)
    exec_wall_ns = int((_time.time() - t0) * 1e9)

    orig, core_of, row, col = ht["out_map"]
    per_core = np.stack([results[c]["out"] for c in range(n_cores)])
    out = np.empty(E, np.float32)
    out[orig] = per_core[core_of, row, col]

    class _Res:
        pass
    res = _Res()
    res.results = results
    res.exec_time_ns = None
    res.exec_wall_ns = exec_wall_ns
    kernel._last_results = res
    return out
